# revision 6
# baseline (speedup 1.0000x reference)
"""Multi-head causal self-attention (B=2, T=2048, D=2048, H=16) on 8 Trainium2
NeuronCores.

Sharding: core c handles batch b = c//4 and 4 heads hs = 4*(c%4) .. hs+4
(batch x tensor-parallel heads). Each core computes Q/K/V projections for its
head slice, causal attention for its 4 heads, and a row-parallel partial of the
output projection (out_heads_slice @ wo_slice.T). The 4 partials per batch are
summed on the host (wo row-parallel reduce); bo is added on one core per batch.

Device layout notes (all matmuls contract over the partition dim, out = lhsT.T @ rhs):
 - x is fed pre-transposed per batch: xT [D, T] so projections can use it as
   the moving operand.
 - Q, K are produced transposed: QT/KT [hd, t] (hd on partitions); scores are
   computed transposed as ST[k, q] = (K-block)^T-matmul, softmax runs WITHOUT
   max subtraction (scores are O(10) here, exp is safe in f32) and the row sums
   come for free as an extra ones-column appended to V in the P~V matmul.
 - V is produced transposed (VT [hd, t]) then PE-transposed into V' [t, hd+1]
   with a ones column.
 - P~ = exp(scale * ST) is masked only on diagonal 128x128 blocks (multiply by
   an upper-triangular 0/1 tile); blocks entirely above the causal diagonal are
   simply never computed or accumulated.
 - O = P~ @ V' lands naturally as [q, hd | rowsum]; normalize by the reciprocal
   of the rowsum column, PE-transpose to OT [hd, q] for the final projection.

All matmul inputs are bf16 (PSUM accumulation in f32).
"""

import sys
import numpy as np

if '/opt/trn_rl_repo' not in sys.path:
    sys.path.insert(0, '/opt/trn_rl_repo')

import ml_dtypes
from contextlib import ExitStack

import concourse.bass as bass
import concourse.mybir as mybir
import concourse.tile as tile
from concourse import bacc
from concourse.bass_utils import run_bass_kernel_spmd

B, T, D, H = 2, 2048, 2048, 16
HD = 128           # head dim
P = 128            # partitions
HPC = 4            # heads per core
NCORES = 8
SCALE = float(HD) ** -0.5
DC = D // P        # 16 contraction chunks for projections
NT = T // P        # 16 t-chunks of 128
QT_TILES = T // 512  # 4 q tiles of 512

BF16 = mybir.dt.bfloat16
F32 = mybir.dt.float32
NPBF16 = ml_dtypes.bfloat16

_BUILD_CACHE = {}


def _build(causal: bool):
    """Build the per-core Bass program (identical across cores; data differs)."""
    nc = bacc.Bacc("TRN2", target_bir_lowering=False, debug=False)

    xT = nc.dram_tensor("xT", [D, T], BF16, kind="ExternalInput").ap()
    wqT = nc.dram_tensor("wqT", [D, HPC * HD], BF16, kind="ExternalInput").ap()
    wkT = nc.dram_tensor("wkT", [D, HPC * HD], BF16, kind="ExternalInput").ap()
    wvT = nc.dram_tensor("wvT", [D, HPC * HD], BF16, kind="ExternalInput").ap()
    woT = nc.dram_tensor("woT", [HPC * HD, D], BF16, kind="ExternalInput").ap()
    bq = nc.dram_tensor("bq", [P, HPC], F32, kind="ExternalInput").ap()
    bk = nc.dram_tensor("bk", [P, HPC], F32, kind="ExternalInput").ap()
    bv = nc.dram_tensor("bv", [P, HPC], F32, kind="ExternalInput").ap()
    bo = nc.dram_tensor("bo", [P, D], F32, kind="ExternalInput").ap()
    tri = nc.dram_tensor("tri", [P, P], BF16, kind="ExternalInput").ap()
    ident = nc.dram_tensor("ident", [P, P], BF16, kind="ExternalInput").ap()
    out = nc.dram_tensor("out", [T, D], F32, kind="ExternalOutput").ap()

    with tile.TileContext(nc) as tc:
        with ExitStack() as ctx:
            persist = ctx.enter_context(tc.tile_pool(name="persist", bufs=1))

            # Q/K weights first, split per contraction chunk, so phase A's
            # first matmuls only wait on the d=0 slices.
            wq_sb = persist.tile([P, DC, HPC * HD], BF16, name="wq_sb")
            wk_sb = persist.tile([P, DC, HPC * HD], BF16, name="wk_sb")
            wv_sb = persist.tile([P, DC, HPC * HD], BF16, name="wv_sb")
            for d in range(DC):
                nc.gpsimd.dma_start(wq_sb[:, d, :], wqT[d * P:(d + 1) * P, :])
                nc.gpsimd.dma_start(wk_sb[:, d, :], wkT[d * P:(d + 1) * P, :])
            for d in range(DC):
                nc.gpsimd.dma_start(wv_sb[:, d, :], wvT[d * P:(d + 1) * P, :])
            bq_sb = persist.tile([P, HPC], F32, name="bq_sb")
            nc.gpsimd.dma_start(bq_sb[:], bq[:])
            bk_sb = persist.tile([P, HPC], F32, name="bk_sb")
            nc.gpsimd.dma_start(bk_sb[:], bk[:])
            tri_sb = persist.tile([P, P], BF16, name="tri_sb")
            nc.gpsimd.dma_start(tri_sb[:], tri[:])
            id_sb = persist.tile([P, P], BF16, name="id_sb")
            nc.gpsimd.dma_start(id_sb[:], ident[:])
            # weights/biases needed only in the merged phase
            wo_sb = persist.tile([P, HPC, D], BF16, name="wo_sb")
            bv_sb = persist.tile([P, HPC], F32, name="bv_sb")
            bo_sb = persist.tile([P, D], F32, name="bo_sb")

            QT_sb = persist.tile([P, HPC, T], BF16, name="QT_sb")
            KT_sb = persist.tile([P, HPC, T], BF16, name="KT_sb")
            # V' with ones column: [t-within-chunk, head, t-chunk, hd+1]
            VP_sb = persist.tile([P, HPC, NT, HD + 1], BF16, name="VP_sb")
            OT_sb = persist.tile([P, HPC, NT, P], BF16, name="OT_sb")

            nc.gpsimd.memset(VP_sb[:, :, :, HD:HD + 1], 1.0)

            # ---- Phase A: Q & K projections (transposed: [hd, t]) ----
            with ExitStack() as pa:
                ax = pa.enter_context(tc.tile_pool(name="ax", bufs=18))
                aps = pa.enter_context(tc.tile_pool(name="aps", bufs=8, space="PSUM"))
                for t4 in range(QT_TILES):
                    xas = []
                    psq = [aps.tile([P, 512], F32, tag="qk", name=f"psq{t4}_{h}")
                           for h in range(HPC)]
                    for d in range(DC):
                        xa = ax.tile([P, 512], BF16, tag="xa", name=f"xa{t4}_{d}")
                        nc.sync.dma_start(xa[:], xT[d * P:(d + 1) * P, t4 * 512:(t4 + 1) * 512])
                        xas.append(xa)
                        st, sp = (d == 0), (d == DC - 1)
                        for h in range(HPC):
                            nc.tensor.matmul(psq[h][:], wq_sb[:, d, h * HD:(h + 1) * HD], xa[:], start=st, stop=sp)
                    for h in range(HPC):
                        nc.vector.tensor_scalar_add(QT_sb[:, h, t4 * 512:(t4 + 1) * 512], psq[h][:], bq_sb[:, h:h + 1])
                    psk = [aps.tile([P, 512], F32, tag="qk", name=f"psk{t4}_{h}")
                           for h in range(HPC)]
                    for d in range(DC):
                        st, sp = (d == 0), (d == DC - 1)
                        for h in range(HPC):
                            nc.tensor.matmul(psk[h][:], wk_sb[:, d, h * HD:(h + 1) * HD], xas[d][:], start=st, stop=sp)
                    for h in range(HPC):
                        nc.vector.tensor_scalar_add(KT_sb[:, h, t4 * 512:(t4 + 1) * 512], psk[h][:], bk_sb[:, h:h + 1])

            # ---- Merged phase: per q-tile stream the V projection for its
            #      4 t-chunks, run attention for its 4 heads, then the
            #      output-projection partial for the finished q rows ----
            with ExitStack() as pc:
                for hh in range(HPC):
                    nc.gpsimd.dma_start(wo_sb[:, hh, :], woT[hh * P:(hh + 1) * P, :])
                nc.gpsimd.dma_start(bv_sb[:], bv[:])
                nc.gpsimd.dma_start(bo_sb[:], bo[:])
                cx = pc.enter_context(tc.tile_pool(name="cx", bufs=18))
                cvt = pc.enter_context(tc.tile_pool(name="cvt", bufs=2))
                cpt = pc.enter_context(tc.tile_pool(name="cpt", bufs=18))
                csm = pc.enter_context(tc.tile_pool(name="csm", bufs=4))
                cob = pc.enter_context(tc.tile_pool(name="cob", bufs=3))
                cps = pc.enter_context(tc.tile_pool(name="cps", bufs=1, space="PSUM"))

                for t4 in range(QT_TILES):
                    # V projection for t-chunk t4 (one head at a time; xb
                    # tiles DMA'd once and reused across heads)
                    xbs = []
                    for h in range(HPC):
                        psv = cps.tile([P, 512], F32, tag="v", bufs=1, name=f"psv{t4}_{h}")
                        for d in range(DC):
                            if h == 0:
                                xb = cx.tile([P, 512], BF16, tag="xb", name=f"xb{t4}_{d}")
                                nc.sync.dma_start(xb[:], xT[d * P:(d + 1) * P, t4 * 512:(t4 + 1) * 512])
                                xbs.append(xb)
                            nc.tensor.matmul(psv[:], wv_sb[:, d, h * HD:(h + 1) * HD], xbs[d][:],
                                             start=(d == 0), stop=(d == DC - 1))
                        vt = cvt.tile([P, 512], BF16, tag="vt", name=f"vt{t4}_{h}")
                        nc.vector.tensor_scalar_add(vt[:], psv[:], bv_sb[:, h:h + 1])
                        for j in range(4):
                            tp = cps.tile([P, P], BF16, tag="tp", bufs=1, name=f"tpv{t4}_{h}_{j}")
                            nc.tensor.transpose(tp[:], vt[:, j * P:(j + 1) * P], id_sb[:])
                            nc.vector.tensor_copy(out=VP_sb[:, h, t4 * 4 + j, 0:HD], in_=tp[:])

                    kmax = 4 * t4 + 4 if causal else NT
                    for h in range(HPC):
                        # scores (transposed) + exp -> P~ tiles
                        pts = []
                        qoffs = []
                        for kb in range(kmax):
                            qoff = max(0, kb - 4 * t4) * P if causal else 0
                            w = 512 - qoff
                            stp = cps.tile([P, 512], F32, tag="st", bufs=2, name=f"st{t4}_{h}_{kb}")
                            nc.tensor.matmul(stp[:, 0:w], KT_sb[:, h, kb * P:(kb + 1) * P],
                                             QT_sb[:, h, t4 * 512 + qoff:(t4 + 1) * 512],
                                             start=True, stop=True)
                            pt = cpt.tile([P, 512], BF16, tag="pt", name=f"pt{t4}_{h}_{kb}")
                            nc.scalar.activation(pt[:, 0:w], stp[:, 0:w],
                                                 mybir.ActivationFunctionType.Exp, scale=SCALE)
                            if causal and kb >= 4 * t4:
                                nc.vector.tensor_mul(out=pt[:, 0:P], in0=pt[:, 0:P], in1=tri_sb[:])
                            pts.append(pt)
                            qoffs.append(qoff)
                        # P~ @ [V | 1] accumulated over k blocks, then normalize
                        for qs in range(4):
                            qb = 4 * t4 + qs
                            klim = qb + 1 if causal else NT
                            ops = cps.tile([P, HD + 1], F32, tag="o", bufs=2, name=f"o{t4}_{h}_{qs}")
                            for kb in range(klim):
                                c0 = qs * P - qoffs[kb]
                                nc.tensor.matmul(ops[:], pts[kb][:, c0:c0 + P],
                                                 VP_sb[:, h, kb, :],
                                                 start=(kb == 0), stop=(kb == klim - 1))
                            rec = csm.tile([P, 1], F32, tag="rec", name=f"rec{t4}_{h}_{qs}")
                            nc.vector.reciprocal(rec[:], ops[:, HD:HD + 1])
                            osb = csm.tile([P, HD], BF16, tag="osb", name=f"osb{t4}_{h}_{qs}")
                            nc.vector.tensor_scalar_mul(osb[:], ops[:, 0:HD], rec[:])
                            tp2 = cps.tile([P, P], BF16, tag="tp", bufs=1, name=f"tpo{t4}_{h}_{qs}")
                            nc.tensor.transpose(tp2[:], osb[:], id_sb[:])
                            nc.vector.tensor_copy(out=OT_sb[:, h, qb, :], in_=tp2[:])
                    # output projection partial for the 4 finished q-chunks
                    for qs in range(4):
                        tch = 4 * t4 + qs
                        for n in range(4):
                            fin = cps.tile([P, 512], F32, tag="fin", bufs=2, name=f"fin{t4}_{qs}_{n}")
                            for h in range(HPC):
                                nc.tensor.matmul(fin[:], OT_sb[:, h, tch, :],
                                                 wo_sb[:, h, n * 512:(n + 1) * 512],
                                                 start=(h == 0), stop=(h == HPC - 1))
                            ob = cob.tile([P, 512], F32, tag="ob", name=f"ob{t4}_{qs}_{n}")
                            nc.vector.tensor_add(out=ob[:], in0=fin[:], in1=bo_sb[:, n * 512:(n + 1) * 512])
                            nc.gpsimd.dma_start(out[tch * P:(tch + 1) * P, n * 512:(n + 1) * 512], ob[:])

    nc.compile()
    return nc


def _get_program(causal: bool):
    if causal not in _BUILD_CACHE:
        _BUILD_CACHE[causal] = _build(causal)
    return _BUILD_CACHE[causal]


def _prep_in_maps(x, wq, bq, wk, bk, wv, bv, wo, bo):
    xbf = np.asarray(x, dtype=np.float32).astype(NPBF16)
    tri = np.triu(np.ones((P, P), dtype=np.float32)).astype(NPBF16)
    ident = np.eye(P, dtype=np.float32).astype(NPBF16)
    wqbf = np.asarray(wq, dtype=np.float32).astype(NPBF16)
    wkbf = np.asarray(wk, dtype=np.float32).astype(NPBF16)
    wvbf = np.asarray(wv, dtype=np.float32).astype(NPBF16)
    wobf = np.asarray(wo, dtype=np.float32).astype(NPBF16)
    bo_bc = np.broadcast_to(np.asarray(bo, np.float32), (P, D)).copy()
    zeros_bc = np.zeros((P, D), np.float32)

    in_maps = []
    for c in range(NCORES):
        b = c // 4
        hs = HPC * HD * (c % 4)
        sl = slice(hs, hs + HPC * HD)
        in_maps.append({
            "xT": np.ascontiguousarray(xbf[b].T),
            "wqT": np.ascontiguousarray(wqbf[sl, :].T),
            "wkT": np.ascontiguousarray(wkbf[sl, :].T),
            "wvT": np.ascontiguousarray(wvbf[sl, :].T),
            "woT": np.ascontiguousarray(wobf[:, sl].T),
            "bq": np.ascontiguousarray(np.asarray(bq, np.float32)[sl].reshape(HPC, P).T),
            "bk": np.ascontiguousarray(np.asarray(bk, np.float32)[sl].reshape(HPC, P).T),
            "bv": np.ascontiguousarray(np.asarray(bv, np.float32)[sl].reshape(HPC, P).T),
            "bo": bo_bc if c % 4 == 0 else zeros_bc,
            "tri": tri,
            "ident": ident,
        })
    return in_maps


def _classify_mask(mask):
    m = np.asarray(mask, dtype=np.float32).reshape(T, T)
    neg = np.isneginf(m)
    if not neg.any():
        return "full"
    if np.array_equal(neg, np.triu(np.ones((T, T), dtype=bool), k=1)):
        return "causal"
    return "other"


def _numpy_reference(x, mask, wq, bq, wk, bk, wv, bv, wo, bo):
    """Fallback for masks that are neither causal nor empty."""
    x = np.asarray(x, np.float32)
    m = np.asarray(mask, np.float32).reshape(T, T)
    q = (x.reshape(-1, D) @ np.asarray(wq, np.float32).T + bq).reshape(B, T, H, HD).transpose(0, 2, 1, 3)
    k = (x.reshape(-1, D) @ np.asarray(wk, np.float32).T + bk).reshape(B, T, H, HD).transpose(0, 2, 1, 3)
    v = (x.reshape(-1, D) @ np.asarray(wv, np.float32).T + bv).reshape(B, T, H, HD).transpose(0, 2, 1, 3)
    outh = np.empty((B, H, T, HD), np.float32)
    negm = np.isneginf(m)
    for b in range(B):
        for h in range(H):
            s = (q[b, h] @ k[b, h].T) * SCALE
            s = np.where(negm, -np.inf, s)
            s = s - s.max(axis=-1, keepdims=True)
            e = np.exp(s)
            p = e / e.sum(axis=-1, keepdims=True)
            outh[b, h] = p @ v[b, h]
    o = outh.transpose(0, 2, 1, 3).reshape(B * T, D)
    return (o @ np.asarray(wo, np.float32).T + bo).reshape(B, T, D).astype(np.float32)


def run_spmd(inputs, trace=False, tmpdir=None):
    """Run the device kernel; returns (output [B,T,D] f32, BassKernelResults)."""
    mode = _classify_mask(inputs["mask"])
    assert mode in ("causal", "full")
    nc = _get_program(mode == "causal")
    in_maps = _prep_in_maps(
        inputs["x"], inputs["wq"], inputs["bq"], inputs["wk"], inputs["bk"],
        inputs["wv"], inputs["bv"], inputs["wo"], inputs["bo"])
    kw = {}
    if trace:
        kw = dict(trace=True, tmpdir=tmpdir)
    res = run_bass_kernel_spmd(nc, in_maps, core_ids=list(range(NCORES)), **kw)
    out = np.empty((B, T, D), np.float32)
    for b in range(B):
        acc = np.zeros((T, D), np.float64)
        for c in range(4 * b, 4 * b + 4):
            acc += res.results[c]["out"].astype(np.float64)
        out[b] = acc.astype(np.float32)
    return out, res


def kernel(**inputs) -> np.ndarray:
    mode = _classify_mask(inputs["mask"])
    if mode == "other":
        return _numpy_reference(**inputs)
    out, _ = run_spmd(inputs)
    return out


# revision 7
# speedup vs baseline: 1.0003x; 1.0003x over previous
"""Multi-head causal self-attention (B=2, T=2048, D=2048, H=16) on 8 Trainium2
NeuronCores.

Sharding: core c handles batch b = c//4 and 4 heads hs = 4*(c%4) .. hs+4
(batch x tensor-parallel heads). Each core computes Q/K/V projections for its
head slice, causal attention for its 4 heads, and a row-parallel partial of the
output projection (out_heads_slice @ wo_slice.T). The 4 partials per batch are
summed on the host (wo row-parallel reduce); bo is added on one core per batch.

Device layout notes (all matmuls contract over the partition dim, out = lhsT.T @ rhs):
 - x is fed pre-transposed per batch: xT [D, T] so projections can use it as
   the moving operand.
 - Q, K are produced transposed: QT/KT [hd, t] (hd on partitions); scores are
   computed transposed as ST[k, q] = (K-block)^T-matmul, softmax runs WITHOUT
   max subtraction (scores are O(10) here, exp is safe in f32) and the row sums
   come for free as an extra ones-column appended to V in the P~V matmul.
 - V is produced transposed (VT [hd, t]) then PE-transposed into V' [t, hd+1]
   with a ones column.
 - P~ = exp(scale * ST) is masked only on diagonal 128x128 blocks (multiply by
   an upper-triangular 0/1 tile); blocks entirely above the causal diagonal are
   simply never computed or accumulated.
 - O = P~ @ V' lands naturally as [q, hd | rowsum]; normalize by the reciprocal
   of the rowsum column, PE-transpose to OT [hd, q] for the final projection.

All matmul inputs are bf16 (PSUM accumulation in f32).
"""

import sys
import numpy as np

if '/opt/trn_rl_repo' not in sys.path:
    sys.path.insert(0, '/opt/trn_rl_repo')

import ml_dtypes
from contextlib import ExitStack

import concourse.bass as bass
import concourse.mybir as mybir
import concourse.tile as tile
from concourse import bacc
from concourse.bass_utils import run_bass_kernel_spmd

B, T, D, H = 2, 2048, 2048, 16
HD = 128           # head dim
P = 128            # partitions
HPC = 4            # heads per core
NCORES = 8
SCALE = float(HD) ** -0.5
DC = D // P        # 16 contraction chunks for projections
NT = T // P        # 16 t-chunks of 128
QT_TILES = T // 512  # 4 q tiles of 512

BF16 = mybir.dt.bfloat16
F32 = mybir.dt.float32
NPBF16 = ml_dtypes.bfloat16

_BUILD_CACHE = {}


def _build(causal: bool):
    """Build the per-core Bass program (identical across cores; data differs)."""
    nc = bacc.Bacc("TRN2", target_bir_lowering=False, debug=False)

    xT = nc.dram_tensor("xT", [D, T], BF16, kind="ExternalInput").ap()
    wqT = nc.dram_tensor("wqT", [D, HPC * HD], BF16, kind="ExternalInput").ap()
    wkT = nc.dram_tensor("wkT", [D, HPC * HD], BF16, kind="ExternalInput").ap()
    wvT = nc.dram_tensor("wvT", [D, HPC * HD], BF16, kind="ExternalInput").ap()
    woT = nc.dram_tensor("woT", [HPC * HD, D], BF16, kind="ExternalInput").ap()
    bq = nc.dram_tensor("bq", [P, HPC], F32, kind="ExternalInput").ap()
    bk = nc.dram_tensor("bk", [P, HPC], F32, kind="ExternalInput").ap()
    bv = nc.dram_tensor("bv", [P, HPC], F32, kind="ExternalInput").ap()
    bo = nc.dram_tensor("bo", [P, D], F32, kind="ExternalInput").ap()
    tri = nc.dram_tensor("tri", [P, P], BF16, kind="ExternalInput").ap()
    ident = nc.dram_tensor("ident", [P, P], BF16, kind="ExternalInput").ap()
    out = nc.dram_tensor("out", [T, D], F32, kind="ExternalOutput").ap()

    with tile.TileContext(nc) as tc:
        with ExitStack() as ctx:
            persist = ctx.enter_context(tc.tile_pool(name="persist", bufs=1))

            # Q/K weights first, split per contraction chunk, so phase A's
            # first matmuls only wait on the d=0 slices.
            wq_sb = persist.tile([P, DC, HPC * HD], BF16, name="wq_sb")
            wk_sb = persist.tile([P, DC, HPC * HD], BF16, name="wk_sb")
            wv_sb = persist.tile([P, DC, HPC * HD], BF16, name="wv_sb")
            for d in range(DC):
                nc.gpsimd.dma_start(wq_sb[:, d, :], wqT[d * P:(d + 1) * P, :])
                nc.gpsimd.dma_start(wk_sb[:, d, :], wkT[d * P:(d + 1) * P, :])
            for d in range(DC):
                nc.gpsimd.dma_start(wv_sb[:, d, :], wvT[d * P:(d + 1) * P, :])
            bq_sb = persist.tile([P, HPC], F32, name="bq_sb")
            nc.gpsimd.dma_start(bq_sb[:], bq[:])
            bk_sb = persist.tile([P, HPC], F32, name="bk_sb")
            nc.gpsimd.dma_start(bk_sb[:], bk[:])
            tri_sb = persist.tile([P, P], BF16, name="tri_sb")
            nc.gpsimd.dma_start(tri_sb[:], tri[:])
            id_sb = persist.tile([P, P], BF16, name="id_sb")
            nc.gpsimd.dma_start(id_sb[:], ident[:])
            # weights/biases needed only in the merged phase
            wo_sb = persist.tile([P, HPC, D], BF16, name="wo_sb")
            bv_sb = persist.tile([P, HPC], F32, name="bv_sb")
            bo_sb = persist.tile([P, D], F32, name="bo_sb")

            QT_sb = persist.tile([P, HPC, T], BF16, name="QT_sb")
            KT_sb = persist.tile([P, HPC, T], BF16, name="KT_sb")
            # V' with ones column: [t-within-chunk, head, t-chunk, hd+1]
            VP_sb = persist.tile([P, HPC, NT, HD + 1], BF16, name="VP_sb")
            OT_sb = persist.tile([P, HPC, NT, P], BF16, name="OT_sb")

            nc.gpsimd.memset(VP_sb[:, :, :, HD:HD + 1], 1.0)

            # ---- Phase A: Q & K projections (transposed: [hd, t]) ----
            with ExitStack() as pa:
                ax = pa.enter_context(tc.tile_pool(name="ax", bufs=18))
                aps = pa.enter_context(tc.tile_pool(name="aps", bufs=4, space="PSUM"))
                for t4 in range(QT_TILES):
                    xas = []
                    psq = [aps.tile([P, 512], F32, tag="q", bufs=4, name=f"psq{t4}_{h}")
                           for h in range(HPC)]
                    for d in range(DC):
                        xa = ax.tile([P, 512], BF16, tag="xa", name=f"xa{t4}_{d}")
                        nc.sync.dma_start(xa[:], xT[d * P:(d + 1) * P, t4 * 512:(t4 + 1) * 512])
                        xas.append(xa)
                        st, sp = (d == 0), (d == DC - 1)
                        for h in range(HPC):
                            nc.tensor.matmul(psq[h][:], wq_sb[:, d, h * HD:(h + 1) * HD], xa[:], start=st, stop=sp)
                    for h in range(HPC):
                        nc.vector.tensor_scalar_add(QT_sb[:, h, t4 * 512:(t4 + 1) * 512], psq[h][:], bq_sb[:, h:h + 1])
                    psk = [aps.tile([P, 512], F32, tag="k", bufs=4, name=f"psk{t4}_{h}")
                           for h in range(HPC)]
                    for d in range(DC):
                        st, sp = (d == 0), (d == DC - 1)
                        for h in range(HPC):
                            nc.tensor.matmul(psk[h][:], wk_sb[:, d, h * HD:(h + 1) * HD], xas[d][:], start=st, stop=sp)
                    for h in range(HPC):
                        nc.vector.tensor_scalar_add(KT_sb[:, h, t4 * 512:(t4 + 1) * 512], psk[h][:], bk_sb[:, h:h + 1])

            # ---- Merged phase: per q-tile stream the V projection for its
            #      4 t-chunks, run attention for its 4 heads, then the
            #      output-projection partial for the finished q rows ----
            with ExitStack() as pc:
                for hh in range(HPC):
                    nc.gpsimd.dma_start(wo_sb[:, hh, :], woT[hh * P:(hh + 1) * P, :])
                nc.gpsimd.dma_start(bv_sb[:], bv[:])
                nc.gpsimd.dma_start(bo_sb[:], bo[:])
                cx = pc.enter_context(tc.tile_pool(name="cx", bufs=18))
                cvt = pc.enter_context(tc.tile_pool(name="cvt", bufs=2))
                cpt = pc.enter_context(tc.tile_pool(name="cpt", bufs=18))
                csm = pc.enter_context(tc.tile_pool(name="csm", bufs=4))
                cob = pc.enter_context(tc.tile_pool(name="cob", bufs=3))
                cps = pc.enter_context(tc.tile_pool(name="cps", bufs=1, space="PSUM"))

                for t4 in range(QT_TILES):
                    # V projection for t-chunk t4 (one head at a time; xb
                    # tiles DMA'd once and reused across heads)
                    xbs = []
                    for h in range(HPC):
                        psv = cps.tile([P, 512], F32, tag="v", bufs=1, name=f"psv{t4}_{h}")
                        for d in range(DC):
                            if h == 0:
                                xb = cx.tile([P, 512], BF16, tag="xb", name=f"xb{t4}_{d}")
                                nc.sync.dma_start(xb[:], xT[d * P:(d + 1) * P, t4 * 512:(t4 + 1) * 512])
                                xbs.append(xb)
                            nc.tensor.matmul(psv[:], wv_sb[:, d, h * HD:(h + 1) * HD], xbs[d][:],
                                             start=(d == 0), stop=(d == DC - 1))
                        vt = cvt.tile([P, 512], BF16, tag="vt", name=f"vt{t4}_{h}")
                        nc.vector.tensor_scalar_add(vt[:], psv[:], bv_sb[:, h:h + 1])
                        for j in range(4):
                            tp = cps.tile([P, P], BF16, tag="tp", bufs=1, name=f"tpv{t4}_{h}_{j}")
                            nc.tensor.transpose(tp[:], vt[:, j * P:(j + 1) * P], id_sb[:])
                            nc.vector.tensor_copy(out=VP_sb[:, h, t4 * 4 + j, 0:HD], in_=tp[:])

                    kmax = 4 * t4 + 4 if causal else NT
                    for h in range(HPC):
                        # scores (transposed) + exp -> P~ tiles
                        pts = []
                        qoffs = []
                        for kb in range(kmax):
                            qoff = max(0, kb - 4 * t4) * P if causal else 0
                            w = 512 - qoff
                            stp = cps.tile([P, 512], F32, tag="st", bufs=2, name=f"st{t4}_{h}_{kb}")
                            nc.tensor.matmul(stp[:, 0:w], KT_sb[:, h, kb * P:(kb + 1) * P],
                                             QT_sb[:, h, t4 * 512 + qoff:(t4 + 1) * 512],
                                             start=True, stop=True)
                            pt = cpt.tile([P, 512], BF16, tag="pt", name=f"pt{t4}_{h}_{kb}")
                            nc.scalar.activation(pt[:, 0:w], stp[:, 0:w],
                                                 mybir.ActivationFunctionType.Exp, scale=SCALE)
                            if causal and kb >= 4 * t4:
                                nc.vector.tensor_mul(out=pt[:, 0:P], in0=pt[:, 0:P], in1=tri_sb[:])
                            pts.append(pt)
                            qoffs.append(qoff)
                        # P~ @ [V | 1] accumulated over k blocks, then normalize
                        for qs in range(4):
                            qb = 4 * t4 + qs
                            klim = qb + 1 if causal else NT
                            ops = cps.tile([P, HD + 1], F32, tag="o", bufs=2, name=f"o{t4}_{h}_{qs}")
                            for kb in range(klim):
                                c0 = qs * P - qoffs[kb]
                                nc.tensor.matmul(ops[:], pts[kb][:, c0:c0 + P],
                                                 VP_sb[:, h, kb, :],
                                                 start=(kb == 0), stop=(kb == klim - 1))
                            rec = csm.tile([P, 1], F32, tag="rec", name=f"rec{t4}_{h}_{qs}")
                            nc.vector.reciprocal(rec[:], ops[:, HD:HD + 1])
                            osb = csm.tile([P, HD], BF16, tag="osb", name=f"osb{t4}_{h}_{qs}")
                            nc.vector.tensor_scalar_mul(osb[:], ops[:, 0:HD], rec[:])
                            tp2 = cps.tile([P, P], BF16, tag="tp", bufs=1, name=f"tpo{t4}_{h}_{qs}")
                            nc.tensor.transpose(tp2[:], osb[:], id_sb[:])
                            nc.vector.tensor_copy(out=OT_sb[:, h, qb, :], in_=tp2[:])
                    # output projection partial for the 4 finished q-chunks
                    for qs in range(4):
                        tch = 4 * t4 + qs
                        for n in range(4):
                            fin = cps.tile([P, 512], F32, tag="fin", bufs=2, name=f"fin{t4}_{qs}_{n}")
                            for h in range(HPC):
                                nc.tensor.matmul(fin[:], OT_sb[:, h, tch, :],
                                                 wo_sb[:, h, n * 512:(n + 1) * 512],
                                                 start=(h == 0), stop=(h == HPC - 1))
                            ob = cob.tile([P, 512], F32, tag="ob", name=f"ob{t4}_{qs}_{n}")
                            nc.vector.tensor_add(out=ob[:], in0=fin[:], in1=bo_sb[:, n * 512:(n + 1) * 512])
                            nc.gpsimd.dma_start(out[tch * P:(tch + 1) * P, n * 512:(n + 1) * 512], ob[:])

    nc.compile()
    return nc


def _get_program(causal: bool):
    if causal not in _BUILD_CACHE:
        _BUILD_CACHE[causal] = _build(causal)
    return _BUILD_CACHE[causal]


def _prep_in_maps(x, wq, bq, wk, bk, wv, bv, wo, bo):
    xbf = np.asarray(x, dtype=np.float32).astype(NPBF16)
    tri = np.triu(np.ones((P, P), dtype=np.float32)).astype(NPBF16)
    ident = np.eye(P, dtype=np.float32).astype(NPBF16)
    wqbf = np.asarray(wq, dtype=np.float32).astype(NPBF16)
    wkbf = np.asarray(wk, dtype=np.float32).astype(NPBF16)
    wvbf = np.asarray(wv, dtype=np.float32).astype(NPBF16)
    wobf = np.asarray(wo, dtype=np.float32).astype(NPBF16)
    bo_bc = np.broadcast_to(np.asarray(bo, np.float32), (P, D)).copy()
    zeros_bc = np.zeros((P, D), np.float32)

    in_maps = []
    for c in range(NCORES):
        b = c // 4
        hs = HPC * HD * (c % 4)
        sl = slice(hs, hs + HPC * HD)
        in_maps.append({
            "xT": np.ascontiguousarray(xbf[b].T),
            "wqT": np.ascontiguousarray(wqbf[sl, :].T),
            "wkT": np.ascontiguousarray(wkbf[sl, :].T),
            "wvT": np.ascontiguousarray(wvbf[sl, :].T),
            "woT": np.ascontiguousarray(wobf[:, sl].T),
            "bq": np.ascontiguousarray(np.asarray(bq, np.float32)[sl].reshape(HPC, P).T),
            "bk": np.ascontiguousarray(np.asarray(bk, np.float32)[sl].reshape(HPC, P).T),
            "bv": np.ascontiguousarray(np.asarray(bv, np.float32)[sl].reshape(HPC, P).T),
            "bo": bo_bc if c % 4 == 0 else zeros_bc,
            "tri": tri,
            "ident": ident,
        })
    return in_maps


def _classify_mask(mask):
    m = np.asarray(mask, dtype=np.float32).reshape(T, T)
    neg = np.isneginf(m)
    if not neg.any():
        return "full"
    if np.array_equal(neg, np.triu(np.ones((T, T), dtype=bool), k=1)):
        return "causal"
    return "other"


def _numpy_reference(x, mask, wq, bq, wk, bk, wv, bv, wo, bo):
    """Fallback for masks that are neither causal nor empty."""
    x = np.asarray(x, np.float32)
    m = np.asarray(mask, np.float32).reshape(T, T)
    q = (x.reshape(-1, D) @ np.asarray(wq, np.float32).T + bq).reshape(B, T, H, HD).transpose(0, 2, 1, 3)
    k = (x.reshape(-1, D) @ np.asarray(wk, np.float32).T + bk).reshape(B, T, H, HD).transpose(0, 2, 1, 3)
    v = (x.reshape(-1, D) @ np.asarray(wv, np.float32).T + bv).reshape(B, T, H, HD).transpose(0, 2, 1, 3)
    outh = np.empty((B, H, T, HD), np.float32)
    negm = np.isneginf(m)
    for b in range(B):
        for h in range(H):
            s = (q[b, h] @ k[b, h].T) * SCALE
            s = np.where(negm, -np.inf, s)
            s = s - s.max(axis=-1, keepdims=True)
            e = np.exp(s)
            p = e / e.sum(axis=-1, keepdims=True)
            outh[b, h] = p @ v[b, h]
    o = outh.transpose(0, 2, 1, 3).reshape(B * T, D)
    return (o @ np.asarray(wo, np.float32).T + bo).reshape(B, T, D).astype(np.float32)


def run_spmd(inputs, trace=False, tmpdir=None):
    """Run the device kernel; returns (output [B,T,D] f32, BassKernelResults)."""
    mode = _classify_mask(inputs["mask"])
    assert mode in ("causal", "full")
    nc = _get_program(mode == "causal")
    in_maps = _prep_in_maps(
        inputs["x"], inputs["wq"], inputs["bq"], inputs["wk"], inputs["bk"],
        inputs["wv"], inputs["bv"], inputs["wo"], inputs["bo"])
    kw = {}
    if trace:
        kw = dict(trace=True, tmpdir=tmpdir)
    res = run_bass_kernel_spmd(nc, in_maps, core_ids=list(range(NCORES)), **kw)
    out = np.empty((B, T, D), np.float32)
    for b in range(B):
        acc = np.zeros((T, D), np.float64)
        for c in range(4 * b, 4 * b + 4):
            acc += res.results[c]["out"].astype(np.float64)
        out[b] = acc.astype(np.float32)
    return out, res


def kernel(**inputs) -> np.ndarray:
    mode = _classify_mask(inputs["mask"])
    if mode == "other":
        return _numpy_reference(**inputs)
    out, _ = run_spmd(inputs)
    return out
